# revision 69
# baseline (speedup 1.0000x reference)
"""DPP attention-3 Trainium2 kernel.

Data-parallel across 8 NeuronCores: one batch element per core; all
weights replicated.  The reference's [B,L,L,L] det_values tensor is never
materialized: since K = s2 @ s2.T is exactly symmetric, the k-reduction of
the 3x3 determinants collapses to

    marginal[i,j] = S0*(d_i d_j - K_ij^2) - d_i a_j - a_i d_j + 2 K_ij A_ij

with  A = K diag(w) K,  a = diag(A),  d = diag(K),  S0 = sum_k w_k d_k.

Everything is computed in the transposed [j, i] orientation (score is
symmetric up to the additive mask, which is fed pre-transposed from the
host), so the exp'd scores are directly the stationary operand of the
context matmul, the softmax denominators fall out of ones-column matmuls,
and the final output projection needs no transpose at all (ctx^T is the
lhsT the natural-orientation matmul wants).

Matmul operands are kept in bf16 (fp32 PSUM accumulation): fp32 matmuls
cost 4 cycles per output row on TRN2 vs 1 for bf16, and the score scale
here (|score| < 1) makes the bf16 rounding negligible (~6e-5 final rel
err measured).  The residual/LayerNorm path stays fp32.
"""

import numpy as np

B, L, H = 8, 160, 64
N_CORES = 8
EPS = 1e-12
CHUNKS = [(0, 128), (128, 32)]  # partition chunks covering L=160

_programs = {}  # (flags..., scale) -> nc


def _build_program(use_mask, use_w, use_bde, use_ln, use_bq, scale):
    import concourse.bass as bass
    import concourse.bacc as bacc_mod
    import concourse.tile as tile
    from concourse import bacc, mybir
    from concourse.masks import make_identity

    f32 = mybir.dt.float32
    bf16 = mybir.dt.bfloat16
    Alu = mybir.AluOpType
    Act = mybir.ActivationFunctionType

    nc = bacc.Bacc(
        "TRN2",
        target_bir_lowering=False,
        debug=False,
        enable_asserts=False,
        num_devices=N_CORES,
    )

    inv_h4 = float(H ** -0.25)

    xt_d = nc.dram_tensor("xt", [H, L], bf16, kind="ExternalInput").ap()
    x_d = nc.dram_tensor("x", [L, H], f32, kind="ExternalInput").ap()
    w3_d = nc.dram_tensor("w3", [H, 3 * H], bf16, kind="ExternalInput").ap()
    bqp_d = None
    if use_bq:
        bqp_d = nc.dram_tensor("bqp", [H, 1], f32, kind="ExternalInput").ap()
    maskt_d = wrow_d = bde_d = lnw_d = lnb_d = None
    if use_mask:
        maskt_d = nc.dram_tensor("maskt", [L, L], f32, kind="ExternalInput").ap()
    if use_w:
        wrow_d = nc.dram_tensor("wrow", [1, L], f32, kind="ExternalInput").ap()
    if use_bde:
        bde_d = nc.dram_tensor("bde", [1, H], f32, kind="ExternalInput").ap()
    if use_ln:
        lnw_d = nc.dram_tensor("lnw", [1, H], f32, kind="ExternalInput").ap()
        lnb_d = nc.dram_tensor("lnb", [1, H], f32, kind="ExternalInput").ap()
    y_d = nc.dram_tensor("y", [L, H], f32, kind="ExternalOutput").ap()

    with tile.TileContext(nc) as tc:
        from contextlib import ExitStack

        with ExitStack() as ctx:
            con = ctx.enter_context(tc.tile_pool(name="con", bufs=1))
            wk = ctx.enter_context(tc.tile_pool(name="wk", bufs=1))
            ppb = ctx.enter_context(tc.tile_pool(name="ppb", bufs=3, space="PSUM"))
            pps = ctx.enter_context(tc.tile_pool(name="pps", bufs=3, space="PSUM"))
            ppm = ctx.enter_context(tc.tile_pool(name="ppm", bufs=2, space="PSUM"))

            # --- inputs; spread descriptor generation across engine DGE
            # queues (a single queue costs ~600ns of issue time per DMA) ---
            # gpsimd's sequencer reaches its first instruction earliest, so
            # the most critical input (xT gates the first matmul) issues there
            xT = con.tile([H, L], bf16)
            nc.gpsimd.dma_start(out=xT[:], in_=xt_d)
            w3 = con.tile([H, 3 * H], bf16)
            nc.sync.dma_start(out=w3[:], in_=w3_d)
            wqt = w3[:, 0:H]
            wvt = w3[:, H : 2 * H]
            wdt = w3[:, 2 * H : 3 * H]
            bqp = con.tile([H, 1], f32)
            if use_bq:
                nc.sync.dma_start(out=bqp[:], in_=bqp_d)
            else:
                nc.vector.memset(bqp[:], 0.0)
            xc = []
            for i, (off, p) in enumerate(CHUNKS):
                t = con.tile([p, H], f32, tag=f"x{off}")
                eng = nc.sync if i == 0 else nc.scalar
                eng.dma_start(out=t[:], in_=x_d[off : off + p, :])
                xc.append(t)

            # --- constants (gpsimd; no deps) ---
            ident_bf = con.tile([128, 128], bf16)
            make_identity(nc, ident_bf[:])
            ones64b = con.tile([H, 1], bf16)
            nc.gpsimd.memset(ones64b[:], 1.0)
            ones128b = con.tile([128, 1], bf16)
            nc.gpsimd.memset(ones128b[:], 1.0)
            onesr = con.tile([1, 128], f32)
            nc.gpsimd.memset(onesr[:], 1.0)
            epsc = con.tile([128, 1], f32)
            nc.gpsimd.memset(epsc[:], EPS)
            ident1 = con.tile([1, 1], f32)
            nc.gpsimd.memset(ident1[:], 1.0)

            masktc = []
            if use_mask:
                for off, p in CHUNKS:
                    t = con.tile([p, L], f32, tag=f"mt{off}")
                    nc.sync.dma_start(out=t[:], in_=maskt_d[off : off + p, :])
                    masktc.append(t)
            if use_w:
                ident = con.tile([128, 128], f32)
                make_identity(nc, ident[:])
                wrow = con.tile([1, L], f32)
                nc.sync.dma_start(out=wrow[:], in_=wrow_d)
            if use_bde:
                bde_r = con.tile([1, H], f32)
                nc.sync.dma_start(out=bde_r[:], in_=bde_d)
            if use_ln:
                lnw_r = con.tile([1, H], f32)
                nc.sync.dma_start(out=lnw_r[:], in_=lnw_d)
                lnb_r = con.tile([1, H], f32)
                nc.sync.dma_start(out=lnb_r[:], in_=lnb_d)

            # pull the single ACT table load off the critical path
            warm = wk.tile([1, 1], f32)
            nc.vector.memset(warm[:], 1.0)
            warm2 = wk.tile([1, 1], f32)
            nc.scalar.copy(warm2[:], warm[:])

            # --- sampler^2 transposed: s2T = Square(invH4*(Wq @ xT) + bq*invH4)
            qT_ps = ppb.tile([H, L], f32, tag="big")
            nc.tensor.matmul(qT_ps[:], wqt, xT[:], start=True, stop=True)
            s2T = wk.tile([H, L], bf16)
            nc.scalar.activation(s2T[:], qT_ps[:], Act.Square, bias=bqp[:], scale=inv_h4)

            # --- K chunks [p, L]; KK taken straight from PSUM so it does not
            # wait on the SBUF cast ---
            Kc = []
            KKc = []
            for i, (off, p) in enumerate(CHUNKS):
                kps = ppb.tile([p, L], f32, tag="big")
                nc.tensor.matmul(kps[:], s2T[:, off : off + p], s2T[:], start=True, stop=True)
                k_sb = wk.tile([p, L], bf16, tag=f"K{off}")
                if i == 0:
                    nc.scalar.copy(k_sb[:], kps[:])
                else:
                    nc.vector.tensor_copy(k_sb[:], kps[:])
                Kc.append(k_sb)
                kk = wk.tile([p, L], bf16, tag=f"KK{off}")
                nc.vector.tensor_mul(kk[:], kps[:], k_sb[:])
                KKc.append(kk)

            # wK2s early on the ACT queue: every A2s matmul needs both chunks
            wK2s = []
            for i, (off, p) in enumerate(CHUNKS):
                t = wk.tile([p, L], bf16, tag=f"wk2{off}")
                if use_w:
                    pass  # filled in after wcol is built
                else:
                    nc.scalar.mul(t[:], Kc[i][:], -2.0 * scale)
                wK2s.append(t)

            # --- d = diag(K) via s4T = s2T*s2T ---
            s4T = wk.tile([H, L], bf16)
            nc.vector.tensor_mul(s4T[:], s2T[:], s2T[:])
            drow_ps = pps.tile([1, L], f32, tag="small")
            nc.tensor.matmul(drow_ps[:], ones64b[:], s4T[:], start=True, stop=True)
            drow = wk.tile([1, L], bf16)
            S0acc_t = wk.tile([1, 1], f32)
            # DVE: the ACT queue is the serializer in this window
            nc.vector.tensor_copy(drow[:], drow_ps[:])
            nc.vector.reduce_sum(S0acc_t[:], drow[:], axis=mybir.AxisListType.X)
            dcol_ps = []
            for off, p in CHUNKS:
                dps = pps.tile([p, 1], f32, tag="small")
                nc.tensor.matmul(dps[:], s4T[:, off : off + p], ones64b[:], start=True, stop=True)
                dcol_ps.append(dps)

            wcol = [None, None]
            if use_w:
                for i, (off, p) in enumerate(CHUNKS):
                    wps = pps.tile([p, 1], f32, tag="small")
                    nc.tensor.transpose(wps[:], wrow[0:1, off : off + p], ident[0:1, 0:1])
                    wc = wk.tile([p, 1], f32, tag=f"wc{off}")
                    nc.vector.tensor_copy(wc[:], wps[:])
                    wcol[i] = wc

            # --- a = diag(K diag(w) K) from KK row sums (emitted early: the
            # a-row -> v0 -> Rs chain gates the final score adds) ---
            if use_w:
                for i in range(2):
                    nc.vector.tensor_scalar(
                        KKc[i][:], KKc[i][:], wcol[i][:], None, op0=Alu.mult
                    )
            arow_ps = pps.tile([1, L], f32, tag="small")
            nc.tensor.matmul(arow_ps[:], ones128b[:], KKc[0][:], start=True, stop=False)
            nc.tensor.matmul(arow_ps[:], ones128b[0:32, :], KKc[1][:], start=False, stop=True)
            arow = wk.tile([1, L], bf16)
            nc.scalar.copy(arow[:], arow_ps[:])

            # S0 = sum_k w_k d_k  (scalar [1,1], fp32)
            if use_w:
                S0_t = wk.tile([1, 1], f32)
                wd_row = wk.tile([1, L], f32)
                nc.vector.tensor_mul(wd_row[:], drow[:], wrow[:])
                nc.vector.reduce_sum(S0_t[:], wd_row[:], axis=mybir.AxisListType.X)
            else:
                S0_t = S0acc_t
            S0s_t = wk.tile([1, 1], f32)
            nc.vector.tensor_scalar(S0s_t[:], S0_t[:], scale, None, op0=Alu.mult)
            nS0s_t = wk.tile([1, 1], f32)
            nc.vector.tensor_scalar(nS0s_t[:], S0_t[:], -scale, None, op0=Alu.mult)
            # scale*S0 broadcast down partition columns (stays in PSUM; the
            # score chain reads it as a per-partition scalar from there)
            S0scol = []
            for off, p in CHUNKS:
                sps = pps.tile([p, 1], f32, tag="small")
                nc.tensor.matmul(sps[:], onesr[0:1, 0:p], S0s_t[:], start=True, stop=True)
                S0scol.append(sps)

            # --- rank-2 factors: v0 = -S0s*d + s*a ; v1 = s*d  (bf16 rows) ---
            v0_r = wk.tile([1, L], bf16)
            tmp_r = wk.tile([1, L], bf16)
            nc.scalar.mul(tmp_r[:], arow[:], scale)
            nc.vector.scalar_tensor_tensor(
                v0_r[:], drow[:], nS0s_t[:], tmp_r[:], op0=Alu.mult, op1=Alu.add
            )
            v1_r = wk.tile([1, L], bf16)
            nc.scalar.mul(v1_r[:], drow[:], scale)

            # diag contribution as a matmul operand:
            # dsel[p, f] = -scale*d_p at f == p+off else 0  (bf16)
            dsel = []
            for i, (off, p) in enumerate(CHUNKS):
                dsc = wk.tile([p, 1], f32, tag=f"dsc{off}")
                nc.scalar.mul(dsc[:], dcol_ps[i][:], -scale)
                ds = wk.tile([p, L], bf16, tag=f"dsel{off}")
                nc.gpsimd.affine_select(
                    out=ds[:],
                    in_=dsc[:, 0:1].broadcast_to([p, L]),
                    compare_op=Alu.is_equal,
                    fill=0.0,
                    base=-off,
                    pattern=[[1, L]],
                    channel_multiplier=-1,
                )
                dsel.append(ds)

            # rank-2 part plus the diagonal (identity-stationary matmul);
            # emitted before the A2s group so it does not gate the final adds
            rsc = []
            for i, (off, p) in enumerate(CHUNKS):
                rs = ppm.tile([p, L], f32, tag="p64")
                nc.tensor.matmul(rs[:], drow[0:1, off : off + p], v0_r[:], start=True, stop=False)
                nc.tensor.matmul(rs[:], arow[0:1, off : off + p], v1_r[:], start=False, stop=False)
                nc.tensor.matmul(rs[:], ident_bf[0:p, 0:p], dsel[i][:], start=False, stop=True)
                rsc.append(rs)

            if use_w:
                for i in range(2):
                    nc.vector.tensor_scalar(
                        wK2s[i][:], Kc[i][:], wcol[i][:], -2.0 * scale,
                        op0=Alu.mult, op1=Alu.mult,
                    )

            # --- score + exp per chunk (transposed orientation) ---
            ec = []
            for i, (off, p) in enumerate(CHUNKS):
                a2s = ppb.tile([p, L], f32, tag="big")
                nc.tensor.matmul(a2s[:], Kc[0][:, off : off + p], wK2s[0][:], start=True, stop=False)
                nc.tensor.matmul(a2s[:], Kc[1][:, off : off + p], wK2s[1][:], start=False, stop=True)

                t1 = wk.tile([p, L], f32, tag=f"t1{off}")
                # t1 = S0s*K + A2s
                nc.vector.scalar_tensor_tensor(
                    t1[:], Kc[i][:], S0scol[i][:], a2s[:], op0=Alu.mult, op1=Alu.add
                )
                # t1 = t1 ⊙ K
                nc.vector.tensor_mul(t1[:], t1[:], Kc[i][:])
                # t1 += Rs + diag
                nc.vector.tensor_add(t1[:], t1[:], rsc[i][:])
                if use_mask:
                    nc.vector.tensor_add(t1[:], t1[:], masktc[i][:])
                e = wk.tile([p, L], bf16, tag=f"e{off}")
                nc.scalar.activation(e[:], t1[:], Act.Exp)
                ec.append(e)

            # --- value projection (bf16) ---
            Vh = []
            for i, (off, p) in enumerate(CHUNKS):
                vps = ppm.tile([p, H], f32, tag="p64")
                nc.tensor.matmul(vps[:], xT[:, off : off + p], wvt, start=True, stop=True)
                vh = wk.tile([p, H], bf16, tag=f"vh{off}")
                nc.scalar.copy(vh[:], vps[:])
                Vh.append(vh)

            # softmax denominators Z as a row (tiny all-ones stationary),
            # transposed to columns, then per-partition reciprocals (a DVE
            # reciprocal along the free dim would serialize: ~7 cyc/element)
            zrow_ps = pps.tile([1, L], f32, tag="small")
            nc.tensor.matmul(zrow_ps[:], ones128b[:], ec[0][:], start=True, stop=False)
            nc.tensor.matmul(zrow_ps[:], ones128b[0:32, :], ec[1][:], start=False, stop=True)
            zrow = wk.tile([1, L], f32)
            nc.vector.tensor_copy(zrow[:], zrow_ps[:])
            rcol = []
            for off, p in CHUNKS:
                zps = pps.tile([p, 1], f32, tag="small")
                nc.tensor.transpose(zps[:], zrow[0:1, off : off + p], ident1[:])
                rc = wk.tile([p, 1], f32, tag=f"rc{off}")
                nc.vector.reciprocal(rc[:], zps[:])
                rcol.append(rc)

            # ctxT [H, L] = V^T e^T
            ctxT_ps = ppb.tile([H, L], f32, tag="big")
            nc.tensor.matmul(ctxT_ps[:], Vh[0][:], ec[0][:], start=True, stop=False)
            nc.tensor.matmul(ctxT_ps[:], Vh[1][:], ec[1][:], start=False, stop=True)
            ctxT = wk.tile([H, L], bf16)
            # split the copy across engines so chunk 0's output matmul starts
            # early and the halves convert in parallel
            nc.scalar.copy(ctxT[:, 0:128], ctxT_ps[:, 0:128])
            nc.vector.tensor_copy(ctxT[:, 128:160], ctxT_ps[:, 128:160])

            if use_bde:
                bde_ps = ppm.tile([128, H], f32, tag="p64")
                nc.tensor.matmul(bde_ps[:], onesr[:], bde_r[:], start=True, stop=True)
                bde_b = wk.tile([128, H], f32)
                nc.vector.tensor_copy(bde_b[:], bde_ps[:])
            if use_ln:
                lnw_ps = ppm.tile([128, H], f32, tag="p64")
                nc.tensor.matmul(lnw_ps[:], onesr[:], lnw_r[:], start=True, stop=True)
                lnw_b = wk.tile([128, H], f32)
                nc.vector.tensor_copy(lnw_b[:], lnw_ps[:])
                lnb_ps = ppm.tile([128, H], f32, tag="p64")
                nc.tensor.matmul(lnb_ps[:], onesr[:], lnb_r[:], start=True, stop=True)
                lnb_b = wk.tile([128, H], f32)
                nc.vector.tensor_copy(lnb_b[:], lnb_ps[:])

            # --- per chunk: output projection (already natural), normalize,
            # residual, LayerNorm ---
            for i, (off, p) in enumerate(CHUNKS):
                # out_nat[i, h'] = sum_h ctxT[h, i] * WdT[h, h']
                ops = ppm.tile([p, H], f32, tag="p64")
                nc.tensor.matmul(ops[:], ctxT[:, off : off + p], wdt, start=True, stop=True)

                res = wk.tile([p, H], f32, tag=f"res{off}")
                # res = out_nat * r + x
                nc.vector.scalar_tensor_tensor(
                    res[:], ops[:], rcol[i][:], xc[i][:], op0=Alu.mult, op1=Alu.add
                )
                if use_bde:
                    nc.vector.tensor_add(res[:], res[:], bde_b[0:p, :])

                stats = wk.tile([p, 6], f32, tag=f"st{off}")
                nc.vector.bn_stats(stats[:], res[:])
                mv = wk.tile([p, 2], f32, tag=f"mv{off}")
                nc.vector.bn_aggr(mv[:], stats[:])
                # rstd = exp(-0.5*ln(var+eps)); Ln+Exp live in one table set
                lnv = wk.tile([p, 1], f32, tag=f"lnv{off}")
                nc.scalar.activation(lnv[:], mv[:, 1:2], Act.Ln, bias=epsc[0:p, :])
                rstd = wk.tile([p, 1], f32, tag=f"rst{off}")
                nc.scalar.activation(rstd[:], lnv[:], Act.Exp, scale=-0.5)

                y_t = wk.tile([p, H], f32, tag=f"y{off}")
                nc.vector.tensor_scalar(
                    y_t[:], res[:], mv[:, 0:1], rstd[:], op0=Alu.subtract, op1=Alu.mult
                )
                if use_ln:
                    nc.vector.tensor_mul(y_t[:], y_t[:], lnw_b[0:p, :])
                    nc.vector.tensor_add(y_t[:], y_t[:], lnb_b[0:p, :])
                # separate DGE queues so the two output stores issue in parallel
                eng = nc.scalar if i == 0 else nc.sync
                eng.dma_start(out=y_d[off : off + p, :], in_=y_t[:])

    # Compile with the combined Ln+Exp activation-table set preferred, so a
    # single ACT_TABLE_LOAD covers Square/Copy/Exp/Ln (the default greedy
    # selection alternates between the exp-only and ln-only sets: 6 loads,
    # ~7.7us of ACT time).  The set *order* must be preserved — the position
    # in this dict is the act_func_set_id walrus resolves against
    # act_info.json — so instead of reordering, hide this kernel's functions
    # from every other set, forcing the selector onto the combined one at
    # its true index.
    orig_tables = bacc_mod.get_activation_tables
    mine = {Act.Exp, Act.Ln, Act.Square, Act.Copy, Act.Identity}

    def _patched(arch):
        tabs = orig_tables(arch)
        assert "natural_log_exp_and_others" in tabs
        return {
            n: (fs if n == "natural_log_exp_and_others" else fs - mine)
            for n, fs in tabs.items()
        }

    bacc_mod.get_activation_tables = _patched
    try:
        nc.compile()
    finally:
        bacc_mod.get_activation_tables = orig_tables
    return nc


def _prepare(inputs):
    import ml_dtypes

    bf = ml_dtypes.bfloat16
    x = np.ascontiguousarray(np.asarray(inputs["input_tensor"], dtype=np.float32))
    mask = np.ascontiguousarray(np.asarray(inputs["attention_mask"], dtype=np.float32))
    Wq = np.asarray(inputs["Wq"], dtype=np.float32)
    bq = np.asarray(inputs["bq"], dtype=np.float32)
    Wv = np.asarray(inputs["Wv"], dtype=np.float32)
    bv = np.asarray(inputs["bv"], dtype=np.float32)
    Wd = np.asarray(inputs["Wd"], dtype=np.float32)
    bd = np.asarray(inputs["bd"], dtype=np.float32)
    ln_w = np.asarray(inputs["ln_w"], dtype=np.float32)
    ln_b = np.asarray(inputs["ln_b"], dtype=np.float32)
    scale = np.float32(np.asarray(inputs["scale_factor"]).reshape(()))

    use_mask = bool(np.any(mask != 0.0))
    wvals = (mask[:, 0, :] > -10000.0).astype(np.float32)
    use_w = not bool(np.all(wvals == 1.0))
    bde = bd + Wd @ bv  # value bias folded through the output projection
    use_bde = bool(np.any(bde != 0.0))
    use_ln = not (bool(np.all(ln_w == 1.0)) and bool(np.all(ln_b == 0.0)))
    use_bq = bool(np.any(bq != 0.0))

    flags = (use_mask, use_w, use_bde, use_ln, use_bq, float(scale))
    w3 = np.concatenate([Wq.T, Wv.T, Wd.T], axis=1)  # [H, 3H]
    shared = {
        "w3": np.ascontiguousarray(w3).astype(bf),
    }
    if use_bq:
        shared["bqp"] = np.ascontiguousarray((bq * (H ** -0.25)).reshape(H, 1))
    if use_bde:
        shared["bde"] = np.ascontiguousarray(bde.reshape(1, H))
    if use_ln:
        shared["lnw"] = np.ascontiguousarray(ln_w.reshape(1, H))
        shared["lnb"] = np.ascontiguousarray(ln_b.reshape(1, H))

    in_maps = []
    for c in range(N_CORES):
        m = dict(shared)
        m["x"] = np.ascontiguousarray(x[c])
        m["xt"] = np.ascontiguousarray(x[c].T).astype(bf)
        if use_mask:
            m["maskt"] = np.ascontiguousarray(mask[c].T)
        if use_w:
            m["wrow"] = np.ascontiguousarray(wvals[c].reshape(1, L))
        in_maps.append(m)
    return flags, in_maps


def _get_program(flags):
    if flags not in _programs:
        _programs[flags] = _build_program(*flags)
    return _programs[flags]


def kernel(**inputs):
    from concourse.bass_utils import run_bass_kernel_spmd

    flags, in_maps = _prepare(inputs)
    nc = _get_program(flags)
    res = run_bass_kernel_spmd(nc, in_maps, core_ids=list(range(N_CORES)))
    out = np.stack([res.results[c]["y"] for c in range(N_CORES)], axis=0)
    return out.astype(np.float32)


# revision 72
# speedup vs baseline: 1.0193x; 1.0193x over previous
"""DPP attention-3 Trainium2 kernel.

Data-parallel across 8 NeuronCores: one batch element per core; all
weights replicated.  The reference's [B,L,L,L] det_values tensor is never
materialized: since K = s2 @ s2.T is exactly symmetric, the k-reduction of
the 3x3 determinants collapses to

    marginal[i,j] = S0*(d_i d_j - K_ij^2) - d_i a_j - a_i d_j + 2 K_ij A_ij

with  A = K diag(w) K,  a = diag(A),  d = diag(K),  S0 = sum_k w_k d_k.

Everything is computed in the transposed [j, i] orientation (score is
symmetric up to the additive mask, which is fed pre-transposed from the
host), so the exp'd scores are directly the stationary operand of the
context matmul, the softmax denominators fall out of ones-column matmuls,
and the final output projection needs no transpose at all (ctx^T is the
lhsT the natural-orientation matmul wants).

Matmul operands are kept in bf16 (fp32 PSUM accumulation): fp32 matmuls
cost 4 cycles per output row on TRN2 vs 1 for bf16, and the score scale
here (|score| < 1) makes the bf16 rounding negligible (~6e-5 final rel
err measured).  The residual/LayerNorm path stays fp32.
"""

import numpy as np

B, L, H = 8, 160, 64
N_CORES = 8
EPS = 1e-12
CHUNKS = [(0, 128), (128, 32)]  # partition chunks covering L=160

_programs = {}  # (flags..., scale) -> nc


def _build_program(use_mask, use_w, use_bde, use_ln, use_bq, scale):
    import concourse.bass as bass
    import concourse.bacc as bacc_mod
    import concourse.tile as tile
    from concourse import bacc, mybir
    from concourse.masks import make_identity

    f32 = mybir.dt.float32
    bf16 = mybir.dt.bfloat16
    Alu = mybir.AluOpType
    Act = mybir.ActivationFunctionType

    nc = bacc.Bacc(
        "TRN2",
        target_bir_lowering=False,
        debug=False,
        enable_asserts=False,
        num_devices=N_CORES,
    )

    inv_h4 = float(H ** -0.25)

    xt_d = nc.dram_tensor("xt", [H, L], bf16, kind="ExternalInput").ap()
    x_d = nc.dram_tensor("x", [L, H], f32, kind="ExternalInput").ap()
    w3_d = nc.dram_tensor("w3", [H, 3 * H], bf16, kind="ExternalInput").ap()
    bqp_d = None
    if use_bq:
        bqp_d = nc.dram_tensor("bqp", [H, 1], f32, kind="ExternalInput").ap()
    maskt_d = wrow_d = bde_d = lnw_d = lnb_d = None
    if use_mask:
        maskt_d = nc.dram_tensor("maskt", [L, L], f32, kind="ExternalInput").ap()
    if use_w:
        wrow_d = nc.dram_tensor("wrow", [1, L], f32, kind="ExternalInput").ap()
    if use_bde:
        bde_d = nc.dram_tensor("bde", [1, H], f32, kind="ExternalInput").ap()
    if use_ln:
        lnw_d = nc.dram_tensor("lnw", [1, H], f32, kind="ExternalInput").ap()
        lnb_d = nc.dram_tensor("lnb", [1, H], f32, kind="ExternalInput").ap()
    y_d = nc.dram_tensor("y", [L, H], f32, kind="ExternalOutput").ap()

    with tile.TileContext(nc) as tc:
        from contextlib import ExitStack

        with ExitStack() as ctx:
            con = ctx.enter_context(tc.tile_pool(name="con", bufs=1))
            wk = ctx.enter_context(tc.tile_pool(name="wk", bufs=1))
            ppb = ctx.enter_context(tc.tile_pool(name="ppb", bufs=3, space="PSUM"))
            pps = ctx.enter_context(tc.tile_pool(name="pps", bufs=3, space="PSUM"))
            ppm = ctx.enter_context(tc.tile_pool(name="ppm", bufs=2, space="PSUM"))

            # --- inputs; spread descriptor generation across engine DGE
            # queues (a single queue costs ~600ns of issue time per DMA) ---
            xT = con.tile([H, L], bf16)
            nc.scalar.dma_start(out=xT[:], in_=xt_d)
            w3 = con.tile([H, 3 * H], bf16)
            nc.sync.dma_start(out=w3[:], in_=w3_d)
            wqt = w3[:, 0:H]
            wvt = w3[:, H : 2 * H]
            wdt = w3[:, 2 * H : 3 * H]
            bqp = con.tile([H, 1], f32)
            if use_bq:
                nc.sync.dma_start(out=bqp[:], in_=bqp_d)
            else:
                nc.vector.memset(bqp[:], 0.0)
            xc = []
            for i, (off, p) in enumerate(CHUNKS):
                t = con.tile([p, H], f32, tag=f"x{off}")
                eng = nc.sync if i == 0 else nc.gpsimd
                eng.dma_start(out=t[:], in_=x_d[off : off + p, :])
                xc.append(t)

            # --- constants (gpsimd; no deps) ---
            ident_bf = con.tile([128, 128], bf16)
            make_identity(nc, ident_bf[:])
            ones64b = con.tile([H, 1], bf16)
            nc.gpsimd.memset(ones64b[:], 1.0)
            ones128b = con.tile([128, 1], bf16)
            nc.gpsimd.memset(ones128b[:], 1.0)
            onesr = con.tile([1, 128], f32)
            nc.gpsimd.memset(onesr[:], 1.0)
            epsc = con.tile([128, 1], f32)
            nc.gpsimd.memset(epsc[:], EPS)
            ident1 = con.tile([1, 1], f32)
            nc.gpsimd.memset(ident1[:], 1.0)

            masktc = []
            if use_mask:
                for off, p in CHUNKS:
                    t = con.tile([p, L], f32, tag=f"mt{off}")
                    nc.sync.dma_start(out=t[:], in_=maskt_d[off : off + p, :])
                    masktc.append(t)
            if use_w:
                ident = con.tile([128, 128], f32)
                make_identity(nc, ident[:])
                wrow = con.tile([1, L], f32)
                nc.sync.dma_start(out=wrow[:], in_=wrow_d)
            if use_bde:
                bde_r = con.tile([1, H], f32)
                nc.sync.dma_start(out=bde_r[:], in_=bde_d)
            if use_ln:
                lnw_r = con.tile([1, H], f32)
                nc.sync.dma_start(out=lnw_r[:], in_=lnw_d)
                lnb_r = con.tile([1, H], f32)
                nc.sync.dma_start(out=lnb_r[:], in_=lnb_d)

            # pull the single ACT table load off the critical path
            warm = wk.tile([1, 1], f32)
            nc.vector.memset(warm[:], 1.0)
            warm2 = wk.tile([1, 1], f32)
            nc.scalar.copy(warm2[:], warm[:])

            # --- sampler^2 transposed: s2T = Square(invH4*(Wq @ xT) + bq*invH4)
            qT_ps = ppb.tile([H, L], f32, tag="big")
            nc.tensor.matmul(qT_ps[:], wqt, xT[:], start=True, stop=True)
            s2T = wk.tile([H, L], bf16)
            nc.scalar.activation(s2T[:], qT_ps[:], Act.Square, bias=bqp[:], scale=inv_h4)

            # --- K chunks [p, L]; KK taken straight from PSUM so it does not
            # wait on the SBUF cast ---
            Kc = []
            KKc = []
            for i, (off, p) in enumerate(CHUNKS):
                kps = ppb.tile([p, L], f32, tag="big")
                nc.tensor.matmul(kps[:], s2T[:, off : off + p], s2T[:], start=True, stop=True)
                k_sb = wk.tile([p, L], bf16, tag=f"K{off}")
                if i == 0:
                    nc.scalar.copy(k_sb[:], kps[:])
                else:
                    nc.vector.tensor_copy(k_sb[:], kps[:])
                Kc.append(k_sb)
                kk = wk.tile([p, L], bf16, tag=f"KK{off}")
                nc.vector.tensor_mul(kk[:], kps[:], k_sb[:])
                KKc.append(kk)

            # wK2s early on the ACT queue: every A2s matmul needs both chunks
            wK2s = []
            for i, (off, p) in enumerate(CHUNKS):
                t = wk.tile([p, L], bf16, tag=f"wk2{off}")
                if use_w:
                    pass  # filled in after wcol is built
                else:
                    nc.scalar.mul(t[:], Kc[i][:], -2.0 * scale)
                wK2s.append(t)

            # --- d = diag(K) via s4T = s2T*s2T ---
            s4T = wk.tile([H, L], bf16)
            nc.vector.tensor_mul(s4T[:], s2T[:], s2T[:])
            drow_ps = pps.tile([1, L], f32, tag="small")
            nc.tensor.matmul(drow_ps[:], ones64b[:], s4T[:], start=True, stop=True)
            drow = wk.tile([1, L], bf16)
            S0acc_t = wk.tile([1, 1], f32)
            # ACT copy-cast with accum_out gives S0 = sum_k d_k for free
            nc.scalar.activation(
                drow[:], drow_ps[:], Act.Copy, accum_out=S0acc_t[:]
            )
            dcol_ps = []
            for off, p in CHUNKS:
                dps = pps.tile([p, 1], f32, tag="small")
                nc.tensor.matmul(dps[:], s4T[:, off : off + p], ones64b[:], start=True, stop=True)
                dcol_ps.append(dps)

            wcol = [None, None]
            if use_w:
                for i, (off, p) in enumerate(CHUNKS):
                    wps = pps.tile([p, 1], f32, tag="small")
                    nc.tensor.transpose(wps[:], wrow[0:1, off : off + p], ident[0:1, 0:1])
                    wc = wk.tile([p, 1], f32, tag=f"wc{off}")
                    nc.vector.tensor_copy(wc[:], wps[:])
                    wcol[i] = wc

            # --- a = diag(K diag(w) K) from KK row sums (emitted early: the
            # a-row -> v0 -> Rs chain gates the final score adds) ---
            if use_w:
                for i in range(2):
                    nc.vector.tensor_scalar(
                        KKc[i][:], KKc[i][:], wcol[i][:], None, op0=Alu.mult
                    )
            arow_ps = pps.tile([1, L], f32, tag="small")
            nc.tensor.matmul(arow_ps[:], ones128b[:], KKc[0][:], start=True, stop=False)
            nc.tensor.matmul(arow_ps[:], ones128b[0:32, :], KKc[1][:], start=False, stop=True)
            arow = wk.tile([1, L], bf16)
            nc.scalar.copy(arow[:], arow_ps[:])

            # S0 = sum_k w_k d_k  (scalar [1,1], fp32)
            if use_w:
                S0_t = wk.tile([1, 1], f32)
                wd_row = wk.tile([1, L], f32)
                nc.vector.tensor_mul(wd_row[:], drow[:], wrow[:])
                nc.vector.reduce_sum(S0_t[:], wd_row[:], axis=mybir.AxisListType.X)
            else:
                S0_t = S0acc_t
            S0s_t = wk.tile([1, 1], f32)
            nc.vector.tensor_scalar(S0s_t[:], S0_t[:], scale, None, op0=Alu.mult)
            nS0s_t = wk.tile([1, 1], f32)
            nc.vector.tensor_scalar(nS0s_t[:], S0_t[:], -scale, None, op0=Alu.mult)
            # scale*S0 broadcast down partition columns (stays in PSUM; the
            # score chain reads it as a per-partition scalar from there)
            S0scol = []
            for off, p in CHUNKS:
                sps = pps.tile([p, 1], f32, tag="small")
                nc.tensor.matmul(sps[:], onesr[0:1, 0:p], S0s_t[:], start=True, stop=True)
                S0scol.append(sps)

            # --- rank-2 factors: v0 = -S0s*d + s*a ; v1 = s*d  (bf16 rows) ---
            v0_r = wk.tile([1, L], bf16)
            tmp_r = wk.tile([1, L], bf16)
            nc.scalar.mul(tmp_r[:], arow[:], scale)
            nc.vector.scalar_tensor_tensor(
                v0_r[:], drow[:], nS0s_t[:], tmp_r[:], op0=Alu.mult, op1=Alu.add
            )
            v1_r = wk.tile([1, L], bf16)
            nc.scalar.mul(v1_r[:], drow[:], scale)

            # diag contribution as a matmul operand:
            # dsel[p, f] = -scale*d_p at f == p+off else 0  (bf16)
            dsel = []
            for i, (off, p) in enumerate(CHUNKS):
                dsc = wk.tile([p, 1], f32, tag=f"dsc{off}")
                nc.scalar.mul(dsc[:], dcol_ps[i][:], -scale)
                ds = wk.tile([p, L], bf16, tag=f"dsel{off}")
                nc.gpsimd.affine_select(
                    out=ds[:],
                    in_=dsc[:, 0:1].broadcast_to([p, L]),
                    compare_op=Alu.is_equal,
                    fill=0.0,
                    base=-off,
                    pattern=[[1, L]],
                    channel_multiplier=-1,
                )
                dsel.append(ds)

            # rank-2 part plus the diagonal (identity-stationary matmul);
            # emitted before the A2s group so it does not gate the final adds
            rsc = []
            for i, (off, p) in enumerate(CHUNKS):
                rs = ppm.tile([p, L], f32, tag="p64")
                nc.tensor.matmul(rs[:], drow[0:1, off : off + p], v0_r[:], start=True, stop=False)
                nc.tensor.matmul(rs[:], arow[0:1, off : off + p], v1_r[:], start=False, stop=False)
                nc.tensor.matmul(rs[:], ident_bf[0:p, 0:p], dsel[i][:], start=False, stop=True)
                rsc.append(rs)

            if use_w:
                for i in range(2):
                    nc.vector.tensor_scalar(
                        wK2s[i][:], Kc[i][:], wcol[i][:], -2.0 * scale,
                        op0=Alu.mult, op1=Alu.mult,
                    )

            # --- score + exp per chunk (transposed orientation) ---
            ec = []
            for i, (off, p) in enumerate(CHUNKS):
                a2s = ppb.tile([p, L], f32, tag="big")
                nc.tensor.matmul(a2s[:], Kc[0][:, off : off + p], wK2s[0][:], start=True, stop=False)
                nc.tensor.matmul(a2s[:], Kc[1][:, off : off + p], wK2s[1][:], start=False, stop=True)

                t1 = wk.tile([p, L], f32, tag=f"t1{off}")
                # t1 = S0s*K + A2s
                nc.vector.scalar_tensor_tensor(
                    t1[:], Kc[i][:], S0scol[i][:], a2s[:], op0=Alu.mult, op1=Alu.add
                )
                # t1 = t1 ⊙ K
                nc.vector.tensor_mul(t1[:], t1[:], Kc[i][:])
                # t1 += Rs + diag
                nc.vector.tensor_add(t1[:], t1[:], rsc[i][:])
                if use_mask:
                    nc.vector.tensor_add(t1[:], t1[:], masktc[i][:])
                e = wk.tile([p, L], bf16, tag=f"e{off}")
                nc.scalar.activation(e[:], t1[:], Act.Exp)
                ec.append(e)

            # --- value projection (bf16) ---
            Vh = []
            for i, (off, p) in enumerate(CHUNKS):
                vps = ppm.tile([p, H], f32, tag="p64")
                nc.tensor.matmul(vps[:], xT[:, off : off + p], wvt, start=True, stop=True)
                vh = wk.tile([p, H], bf16, tag=f"vh{off}")
                nc.scalar.copy(vh[:], vps[:])
                Vh.append(vh)

            # softmax denominators Z as a row (tiny all-ones stationary),
            # transposed to columns, then per-partition reciprocals (a DVE
            # reciprocal along the free dim would serialize: ~7 cyc/element)
            zrow_ps = pps.tile([1, L], f32, tag="small")
            nc.tensor.matmul(zrow_ps[:], ones128b[:], ec[0][:], start=True, stop=False)
            nc.tensor.matmul(zrow_ps[:], ones128b[0:32, :], ec[1][:], start=False, stop=True)
            zrow = wk.tile([1, L], f32)
            nc.vector.tensor_copy(zrow[:], zrow_ps[:])
            rcol = []
            for off, p in CHUNKS:
                zps = pps.tile([p, 1], f32, tag="small")
                nc.tensor.transpose(zps[:], zrow[0:1, off : off + p], ident1[:])
                rc = wk.tile([p, 1], f32, tag=f"rc{off}")
                nc.vector.reciprocal(rc[:], zps[:])
                rcol.append(rc)

            # ctxT [H, L] = V^T e^T
            ctxT_ps = ppb.tile([H, L], f32, tag="big")
            nc.tensor.matmul(ctxT_ps[:], Vh[0][:], ec[0][:], start=True, stop=False)
            nc.tensor.matmul(ctxT_ps[:], Vh[1][:], ec[1][:], start=False, stop=True)
            ctxT = wk.tile([H, L], bf16)
            # split the copy across engines so chunk 0's output matmul starts
            # early and the halves convert in parallel
            nc.scalar.copy(ctxT[:, 0:128], ctxT_ps[:, 0:128])
            nc.vector.tensor_copy(ctxT[:, 128:160], ctxT_ps[:, 128:160])

            if use_bde:
                bde_ps = ppm.tile([128, H], f32, tag="p64")
                nc.tensor.matmul(bde_ps[:], onesr[:], bde_r[:], start=True, stop=True)
                bde_b = wk.tile([128, H], f32)
                nc.vector.tensor_copy(bde_b[:], bde_ps[:])
            if use_ln:
                lnw_ps = ppm.tile([128, H], f32, tag="p64")
                nc.tensor.matmul(lnw_ps[:], onesr[:], lnw_r[:], start=True, stop=True)
                lnw_b = wk.tile([128, H], f32)
                nc.vector.tensor_copy(lnw_b[:], lnw_ps[:])
                lnb_ps = ppm.tile([128, H], f32, tag="p64")
                nc.tensor.matmul(lnb_ps[:], onesr[:], lnb_r[:], start=True, stop=True)
                lnb_b = wk.tile([128, H], f32)
                nc.vector.tensor_copy(lnb_b[:], lnb_ps[:])

            # --- per chunk: output projection (already natural), normalize,
            # residual, LayerNorm ---
            for i, (off, p) in enumerate(CHUNKS):
                # out_nat[i, h'] = sum_h ctxT[h, i] * WdT[h, h']
                ops = ppm.tile([p, H], f32, tag="p64")
                nc.tensor.matmul(ops[:], ctxT[:, off : off + p], wdt, start=True, stop=True)

                res = wk.tile([p, H], f32, tag=f"res{off}")
                # res = out_nat * r + x
                nc.vector.scalar_tensor_tensor(
                    res[:], ops[:], rcol[i][:], xc[i][:], op0=Alu.mult, op1=Alu.add
                )
                if use_bde:
                    nc.vector.tensor_add(res[:], res[:], bde_b[0:p, :])

                stats = wk.tile([p, 6], f32, tag=f"st{off}")
                nc.vector.bn_stats(stats[:], res[:])
                mv = wk.tile([p, 2], f32, tag=f"mv{off}")
                nc.vector.bn_aggr(mv[:], stats[:])
                # rstd = exp(-0.5*ln(var+eps)); Ln+Exp live in one table set
                lnv = wk.tile([p, 1], f32, tag=f"lnv{off}")
                nc.scalar.activation(lnv[:], mv[:, 1:2], Act.Ln, bias=epsc[0:p, :])
                rstd = wk.tile([p, 1], f32, tag=f"rst{off}")
                nc.scalar.activation(rstd[:], lnv[:], Act.Exp, scale=-0.5)

                y_t = wk.tile([p, H], f32, tag=f"y{off}")
                nc.vector.tensor_scalar(
                    y_t[:], res[:], mv[:, 0:1], rstd[:], op0=Alu.subtract, op1=Alu.mult
                )
                if use_ln:
                    nc.vector.tensor_mul(y_t[:], y_t[:], lnw_b[0:p, :])
                    nc.vector.tensor_add(y_t[:], y_t[:], lnb_b[0:p, :])
                # separate DGE queues so the two output stores issue in parallel
                eng = nc.scalar if i == 0 else nc.sync
                eng.dma_start(out=y_d[off : off + p, :], in_=y_t[:])

    # Compile with the combined Ln+Exp activation-table set preferred, so a
    # single ACT_TABLE_LOAD covers Square/Copy/Exp/Ln (the default greedy
    # selection alternates between the exp-only and ln-only sets: 6 loads,
    # ~7.7us of ACT time).  The set *order* must be preserved — the position
    # in this dict is the act_func_set_id walrus resolves against
    # act_info.json — so instead of reordering, hide this kernel's functions
    # from every other set, forcing the selector onto the combined one at
    # its true index.
    orig_tables = bacc_mod.get_activation_tables
    mine = {Act.Exp, Act.Ln, Act.Square, Act.Copy, Act.Identity}

    def _patched(arch):
        tabs = orig_tables(arch)
        assert "natural_log_exp_and_others" in tabs
        return {
            n: (fs if n == "natural_log_exp_and_others" else fs - mine)
            for n, fs in tabs.items()
        }

    bacc_mod.get_activation_tables = _patched
    try:
        nc.compile()
    finally:
        bacc_mod.get_activation_tables = orig_tables
    return nc


def _prepare(inputs):
    import ml_dtypes

    bf = ml_dtypes.bfloat16
    x = np.ascontiguousarray(np.asarray(inputs["input_tensor"], dtype=np.float32))
    mask = np.ascontiguousarray(np.asarray(inputs["attention_mask"], dtype=np.float32))
    Wq = np.asarray(inputs["Wq"], dtype=np.float32)
    bq = np.asarray(inputs["bq"], dtype=np.float32)
    Wv = np.asarray(inputs["Wv"], dtype=np.float32)
    bv = np.asarray(inputs["bv"], dtype=np.float32)
    Wd = np.asarray(inputs["Wd"], dtype=np.float32)
    bd = np.asarray(inputs["bd"], dtype=np.float32)
    ln_w = np.asarray(inputs["ln_w"], dtype=np.float32)
    ln_b = np.asarray(inputs["ln_b"], dtype=np.float32)
    scale = np.float32(np.asarray(inputs["scale_factor"]).reshape(()))

    use_mask = bool(np.any(mask != 0.0))
    wvals = (mask[:, 0, :] > -10000.0).astype(np.float32)
    use_w = not bool(np.all(wvals == 1.0))
    bde = bd + Wd @ bv  # value bias folded through the output projection
    use_bde = bool(np.any(bde != 0.0))
    use_ln = not (bool(np.all(ln_w == 1.0)) and bool(np.all(ln_b == 0.0)))
    use_bq = bool(np.any(bq != 0.0))

    flags = (use_mask, use_w, use_bde, use_ln, use_bq, float(scale))
    w3 = np.concatenate([Wq.T, Wv.T, Wd.T], axis=1)  # [H, 3H]
    shared = {
        "w3": np.ascontiguousarray(w3).astype(bf),
    }
    if use_bq:
        shared["bqp"] = np.ascontiguousarray((bq * (H ** -0.25)).reshape(H, 1))
    if use_bde:
        shared["bde"] = np.ascontiguousarray(bde.reshape(1, H))
    if use_ln:
        shared["lnw"] = np.ascontiguousarray(ln_w.reshape(1, H))
        shared["lnb"] = np.ascontiguousarray(ln_b.reshape(1, H))

    in_maps = []
    for c in range(N_CORES):
        m = dict(shared)
        m["x"] = np.ascontiguousarray(x[c])
        m["xt"] = np.ascontiguousarray(x[c].T).astype(bf)
        if use_mask:
            m["maskt"] = np.ascontiguousarray(mask[c].T)
        if use_w:
            m["wrow"] = np.ascontiguousarray(wvals[c].reshape(1, L))
        in_maps.append(m)
    return flags, in_maps


def _get_program(flags):
    if flags not in _programs:
        _programs[flags] = _build_program(*flags)
    return _programs[flags]


def kernel(**inputs):
    from concourse.bass_utils import run_bass_kernel_spmd

    flags, in_maps = _prepare(inputs)
    nc = _get_program(flags)
    res = run_bass_kernel_spmd(nc, in_maps, core_ids=list(range(N_CORES)))
    out = np.stack([res.results[c]["y"] for c in range(N_CORES)], axis=0)
    return out.astype(np.float32)


# revision 75
# speedup vs baseline: 1.0374x; 1.0177x over previous
"""DPP attention-3 Trainium2 kernel.

Data-parallel across 8 NeuronCores: one batch element per core; all
weights replicated.  The reference's [B,L,L,L] det_values tensor is never
materialized: since K = s2 @ s2.T is exactly symmetric, the k-reduction of
the 3x3 determinants collapses to

    marginal[i,j] = S0*(d_i d_j - K_ij^2) - d_i a_j - a_i d_j + 2 K_ij A_ij

with  A = K diag(w) K,  a = diag(A),  d = diag(K),  S0 = sum_k w_k d_k.

Everything is computed in the transposed [j, i] orientation (score is
symmetric up to the additive mask, which is fed pre-transposed from the
host), so the exp'd scores are directly the stationary operand of the
context matmul, the softmax denominators fall out of ones-column matmuls,
and the final output projection needs no transpose at all (ctx^T is the
lhsT the natural-orientation matmul wants).

Matmul operands are kept in bf16 (fp32 PSUM accumulation): fp32 matmuls
cost 4 cycles per output row on TRN2 vs 1 for bf16, and the score scale
here (|score| < 1) makes the bf16 rounding negligible (~6e-5 final rel
err measured).  The residual/LayerNorm path stays fp32.
"""

import numpy as np

B, L, H = 8, 160, 64
N_CORES = 8
EPS = 1e-12
CHUNKS = [(0, 128), (128, 32)]  # partition chunks covering L=160

_programs = {}  # (flags..., scale) -> nc


def _build_program(use_mask, use_w, use_bde, use_ln, use_bq, scale):
    import concourse.bass as bass
    import concourse.bacc as bacc_mod
    import concourse.tile as tile
    from concourse import bacc, mybir
    from concourse.masks import make_identity

    f32 = mybir.dt.float32
    bf16 = mybir.dt.bfloat16
    Alu = mybir.AluOpType
    Act = mybir.ActivationFunctionType

    nc = bacc.Bacc(
        "TRN2",
        target_bir_lowering=False,
        debug=False,
        enable_asserts=False,
        num_devices=N_CORES,
    )

    inv_h4 = float(H ** -0.25)

    xt_d = nc.dram_tensor("xt", [H, L], bf16, kind="ExternalInput").ap()
    x_d = nc.dram_tensor("x", [L, H], f32, kind="ExternalInput").ap()
    w3_d = nc.dram_tensor("w3", [H, 3 * H], bf16, kind="ExternalInput").ap()
    bqp_d = None
    if use_bq:
        bqp_d = nc.dram_tensor("bqp", [H, 1], f32, kind="ExternalInput").ap()
    maskt_d = wrow_d = bde_d = lnw_d = lnb_d = None
    if use_mask:
        maskt_d = nc.dram_tensor("maskt", [L, L], f32, kind="ExternalInput").ap()
    if use_w:
        wrow_d = nc.dram_tensor("wrow", [1, L], f32, kind="ExternalInput").ap()
    if use_bde:
        bde_d = nc.dram_tensor("bde", [1, H], f32, kind="ExternalInput").ap()
    if use_ln:
        lnw_d = nc.dram_tensor("lnw", [1, H], f32, kind="ExternalInput").ap()
        lnb_d = nc.dram_tensor("lnb", [1, H], f32, kind="ExternalInput").ap()
    y_d = nc.dram_tensor("y", [L, H], f32, kind="ExternalOutput").ap()

    with tile.TileContext(nc) as tc:
        from contextlib import ExitStack

        with ExitStack() as ctx:
            con = ctx.enter_context(tc.tile_pool(name="con", bufs=1))
            wk = ctx.enter_context(tc.tile_pool(name="wk", bufs=1))
            ppb = ctx.enter_context(tc.tile_pool(name="ppb", bufs=3, space="PSUM"))
            pps = ctx.enter_context(tc.tile_pool(name="pps", bufs=3, space="PSUM"))
            ppm = ctx.enter_context(tc.tile_pool(name="ppm", bufs=2, space="PSUM"))

            # --- inputs; spread descriptor generation across engine DGE
            # queues (a single queue costs ~600ns of issue time per DMA) ---
            xT = con.tile([H, L], bf16)
            nc.scalar.dma_start(out=xT[:], in_=xt_d)
            w3 = con.tile([H, 3 * H], bf16)
            nc.sync.dma_start(out=w3[:], in_=w3_d)
            wqt = w3[:, 0:H]
            wvt = w3[:, H : 2 * H]
            wdt = w3[:, 2 * H : 3 * H]
            bqp = con.tile([H, 1], f32)
            if use_bq:
                nc.sync.dma_start(out=bqp[:], in_=bqp_d)
            else:
                nc.vector.memset(bqp[:], 0.0)
            xc = []
            for i, (off, p) in enumerate(CHUNKS):
                t = con.tile([p, H], f32, tag=f"x{off}")
                eng = nc.sync if i == 0 else nc.gpsimd
                eng.dma_start(out=t[:], in_=x_d[off : off + p, :])
                xc.append(t)

            # --- constants (gpsimd; no deps) ---
            ident_bf = con.tile([128, 128], bf16)
            make_identity(nc, ident_bf[:])
            ones64b = con.tile([H, 1], bf16)
            nc.gpsimd.memset(ones64b[:], 1.0)
            ones128b = con.tile([128, 1], bf16)
            nc.gpsimd.memset(ones128b[:], 1.0)
            onesr = con.tile([1, 128], f32)
            nc.gpsimd.memset(onesr[:], 1.0)
            epsc = con.tile([128, 1], f32)
            nc.gpsimd.memset(epsc[:], EPS)
            ident1 = con.tile([1, 1], f32)
            nc.gpsimd.memset(ident1[:], 1.0)

            masktc = []
            if use_mask:
                for off, p in CHUNKS:
                    t = con.tile([p, L], f32, tag=f"mt{off}")
                    nc.sync.dma_start(out=t[:], in_=maskt_d[off : off + p, :])
                    masktc.append(t)
            if use_w:
                ident = con.tile([128, 128], f32)
                make_identity(nc, ident[:])
                wrow = con.tile([1, L], f32)
                nc.sync.dma_start(out=wrow[:], in_=wrow_d)
            if use_bde:
                bde_r = con.tile([1, H], f32)
                nc.sync.dma_start(out=bde_r[:], in_=bde_d)
            if use_ln:
                lnw_r = con.tile([1, H], f32)
                nc.sync.dma_start(out=lnw_r[:], in_=lnw_d)
                lnb_r = con.tile([1, H], f32)
                nc.sync.dma_start(out=lnb_r[:], in_=lnb_d)

            # pull the single ACT table load off the critical path
            warm = wk.tile([1, 1], f32)
            nc.vector.memset(warm[:], 1.0)
            warm2 = wk.tile([1, 1], f32)
            nc.scalar.copy(warm2[:], warm[:])

            # --- sampler^2 transposed: s2T = Square(invH4*(Wq @ xT) + bq*invH4)
            qT_ps = ppb.tile([H, L], f32, tag="big")
            nc.tensor.matmul(qT_ps[:], wqt, xT[:], start=True, stop=True)
            s2T = wk.tile([H, L], bf16)
            nc.scalar.activation(s2T[:], qT_ps[:], Act.Square, bias=bqp[:], scale=inv_h4)

            # --- K chunks [p, L]; KK taken straight from PSUM so it does not
            # wait on the SBUF cast ---
            Kc = []
            KKc = []
            for i, (off, p) in enumerate(CHUNKS):
                kps = ppb.tile([p, L], f32, tag="big")
                nc.tensor.matmul(kps[:], s2T[:, off : off + p], s2T[:], start=True, stop=True)
                k_sb = wk.tile([p, L], bf16, tag=f"K{off}")
                if i == 0:
                    nc.scalar.copy(k_sb[:], kps[:])
                else:
                    nc.vector.tensor_copy(k_sb[:], kps[:])
                Kc.append(k_sb)
                kk = wk.tile([p, L], bf16, tag=f"KK{off}")
                nc.vector.tensor_mul(kk[:], kps[:], k_sb[:])
                KKc.append(kk)

            # wK2s early on the ACT queue: every A2s matmul needs both chunks
            wK2s = []
            for i, (off, p) in enumerate(CHUNKS):
                t = wk.tile([p, L], bf16, tag=f"wk2{off}")
                if use_w:
                    pass  # filled in after wcol is built
                else:
                    nc.scalar.mul(t[:], Kc[i][:], -2.0 * scale)
                wK2s.append(t)

            # --- d = diag(K) via s4T = s2T*s2T ---
            s4T = wk.tile([H, L], bf16)
            nc.vector.tensor_mul(s4T[:], s2T[:], s2T[:])
            drow_ps = pps.tile([1, L], f32, tag="small")
            nc.tensor.matmul(drow_ps[:], ones64b[:], s4T[:], start=True, stop=True)
            drow = wk.tile([1, L], bf16)
            S0acc_t = wk.tile([1, 1], f32)
            # ACT copy-cast with accum_out gives S0 = sum_k d_k for free
            nc.scalar.activation(
                drow[:], drow_ps[:], Act.Copy, accum_out=S0acc_t[:]
            )
            dcol_ps = []
            for off, p in CHUNKS:
                dps = pps.tile([p, 1], f32, tag="small")
                nc.tensor.matmul(dps[:], s4T[:, off : off + p], ones64b[:], start=True, stop=True)
                dcol_ps.append(dps)

            wcol = [None, None]
            if use_w:
                for i, (off, p) in enumerate(CHUNKS):
                    wps = pps.tile([p, 1], f32, tag="small")
                    nc.tensor.transpose(wps[:], wrow[0:1, off : off + p], ident[0:1, 0:1])
                    wc = wk.tile([p, 1], f32, tag=f"wc{off}")
                    nc.vector.tensor_copy(wc[:], wps[:])
                    wcol[i] = wc

            # --- a = diag(K diag(w) K) from KK row sums (emitted early: the
            # a-row -> v0 -> Rs chain gates the final score adds) ---
            if use_w:
                for i in range(2):
                    nc.vector.tensor_scalar(
                        KKc[i][:], KKc[i][:], wcol[i][:], None, op0=Alu.mult
                    )
            arow_ps = pps.tile([1, L], f32, tag="small")
            nc.tensor.matmul(arow_ps[:], ones128b[:], KKc[0][:], start=True, stop=False)
            nc.tensor.matmul(arow_ps[:], ones128b[0:32, :], KKc[1][:], start=False, stop=True)
            # scale*a directly off PSUM: the raw a-row is never needed —
            # Rs = d (x) v0 + (s*a) (x) d, with v0 = -S0s*d + s*a
            arow_s = wk.tile([1, L], bf16)
            nc.scalar.mul(arow_s[:], arow_ps[:], scale)

            # S0 = sum_k w_k d_k  (scalar [1,1], fp32)
            if use_w:
                S0_t = wk.tile([1, 1], f32)
                wd_row = wk.tile([1, L], f32)
                nc.vector.tensor_mul(wd_row[:], drow[:], wrow[:])
                nc.vector.reduce_sum(S0_t[:], wd_row[:], axis=mybir.AxisListType.X)
            else:
                S0_t = S0acc_t
            S0s_t = wk.tile([1, 1], f32)
            nc.vector.tensor_scalar(S0s_t[:], S0_t[:], scale, None, op0=Alu.mult)
            nS0s_t = wk.tile([1, 1], f32)
            nc.vector.tensor_scalar(nS0s_t[:], S0_t[:], -scale, None, op0=Alu.mult)
            # scale*S0 broadcast down partition columns (stays in PSUM; the
            # score chain reads it as a per-partition scalar from there)
            S0scol = []
            for off, p in CHUNKS:
                sps = pps.tile([p, 1], f32, tag="small")
                nc.tensor.matmul(sps[:], onesr[0:1, 0:p], S0s_t[:], start=True, stop=True)
                S0scol.append(sps)

            # --- rank-2 factor: v0 = -S0s*d + s*a  (bf16 row) ---
            v0_r = wk.tile([1, L], bf16)
            nc.vector.scalar_tensor_tensor(
                v0_r[:], drow[:], nS0s_t[:], arow_s[:], op0=Alu.mult, op1=Alu.add
            )

            # diag contribution as a matmul operand:
            # dsel[p, f] = -scale*d_p at f == p+off else 0  (bf16)
            dsel = []
            for i, (off, p) in enumerate(CHUNKS):
                dsc = wk.tile([p, 1], f32, tag=f"dsc{off}")
                nc.scalar.mul(dsc[:], dcol_ps[i][:], -scale)
                ds = wk.tile([p, L], bf16, tag=f"dsel{off}")
                nc.gpsimd.affine_select(
                    out=ds[:],
                    in_=dsc[:, 0:1].broadcast_to([p, L]),
                    compare_op=Alu.is_equal,
                    fill=0.0,
                    base=-off,
                    pattern=[[1, L]],
                    channel_multiplier=-1,
                )
                dsel.append(ds)

            # rank-2 part plus the diagonal (identity-stationary matmul);
            # emitted before the A2s group so it does not gate the final adds
            rsc = []
            for i, (off, p) in enumerate(CHUNKS):
                rs = ppm.tile([p, L], f32, tag="p64")
                nc.tensor.matmul(rs[:], drow[0:1, off : off + p], v0_r[:], start=True, stop=False)
                nc.tensor.matmul(rs[:], arow_s[0:1, off : off + p], drow[:], start=False, stop=False)
                nc.tensor.matmul(rs[:], ident_bf[0:p, 0:p], dsel[i][:], start=False, stop=True)
                rsc.append(rs)

            if use_w:
                for i in range(2):
                    nc.vector.tensor_scalar(
                        wK2s[i][:], Kc[i][:], wcol[i][:], -2.0 * scale,
                        op0=Alu.mult, op1=Alu.mult,
                    )

            # --- score + exp per chunk (transposed orientation) ---
            ec = []
            for i, (off, p) in enumerate(CHUNKS):
                a2s = ppb.tile([p, L], f32, tag="big")
                nc.tensor.matmul(a2s[:], Kc[0][:, off : off + p], wK2s[0][:], start=True, stop=False)
                nc.tensor.matmul(a2s[:], Kc[1][:, off : off + p], wK2s[1][:], start=False, stop=True)

                t1 = wk.tile([p, L], f32, tag=f"t1{off}")
                # t1 = S0s*K + A2s
                nc.vector.scalar_tensor_tensor(
                    t1[:], Kc[i][:], S0scol[i][:], a2s[:], op0=Alu.mult, op1=Alu.add
                )
                # t1 = t1 ⊙ K
                nc.vector.tensor_mul(t1[:], t1[:], Kc[i][:])
                # t1 += Rs + diag
                nc.vector.tensor_add(t1[:], t1[:], rsc[i][:])
                if use_mask:
                    nc.vector.tensor_add(t1[:], t1[:], masktc[i][:])
                e = wk.tile([p, L], bf16, tag=f"e{off}")
                nc.scalar.activation(e[:], t1[:], Act.Exp)
                ec.append(e)

            # --- value projection (bf16) ---
            Vh = []
            for i, (off, p) in enumerate(CHUNKS):
                vps = ppm.tile([p, H], f32, tag="p64")
                nc.tensor.matmul(vps[:], xT[:, off : off + p], wvt, start=True, stop=True)
                vh = wk.tile([p, H], bf16, tag=f"vh{off}")
                nc.scalar.copy(vh[:], vps[:])
                Vh.append(vh)

            # softmax denominators Z as a row (tiny all-ones stationary),
            # transposed to columns, then per-partition reciprocals (a DVE
            # reciprocal along the free dim would serialize: ~7 cyc/element)
            zrow_ps = pps.tile([1, L], f32, tag="small")
            nc.tensor.matmul(zrow_ps[:], ones128b[:], ec[0][:], start=True, stop=False)
            nc.tensor.matmul(zrow_ps[:], ones128b[0:32, :], ec[1][:], start=False, stop=True)
            zrow = wk.tile([1, L], f32)
            nc.vector.tensor_copy(zrow[:], zrow_ps[:])
            rcol = []
            for off, p in CHUNKS:
                zps = pps.tile([p, 1], f32, tag="small")
                nc.tensor.transpose(zps[:], zrow[0:1, off : off + p], ident1[:])
                rc = wk.tile([p, 1], f32, tag=f"rc{off}")
                nc.vector.reciprocal(rc[:], zps[:])
                rcol.append(rc)

            # ctxT [H, L] = V^T e^T
            ctxT_ps = ppb.tile([H, L], f32, tag="big")
            nc.tensor.matmul(ctxT_ps[:], Vh[0][:], ec[0][:], start=True, stop=False)
            nc.tensor.matmul(ctxT_ps[:], Vh[1][:], ec[1][:], start=False, stop=True)
            ctxT = wk.tile([H, L], bf16)
            # split the copy across engines so chunk 0's output matmul starts
            # early and the halves convert in parallel
            nc.scalar.copy(ctxT[:, 0:128], ctxT_ps[:, 0:128])
            nc.vector.tensor_copy(ctxT[:, 128:160], ctxT_ps[:, 128:160])

            if use_bde:
                bde_ps = ppm.tile([128, H], f32, tag="p64")
                nc.tensor.matmul(bde_ps[:], onesr[:], bde_r[:], start=True, stop=True)
                bde_b = wk.tile([128, H], f32)
                nc.vector.tensor_copy(bde_b[:], bde_ps[:])
            if use_ln:
                lnw_ps = ppm.tile([128, H], f32, tag="p64")
                nc.tensor.matmul(lnw_ps[:], onesr[:], lnw_r[:], start=True, stop=True)
                lnw_b = wk.tile([128, H], f32)
                nc.vector.tensor_copy(lnw_b[:], lnw_ps[:])
                lnb_ps = ppm.tile([128, H], f32, tag="p64")
                nc.tensor.matmul(lnb_ps[:], onesr[:], lnb_r[:], start=True, stop=True)
                lnb_b = wk.tile([128, H], f32)
                nc.vector.tensor_copy(lnb_b[:], lnb_ps[:])

            # --- per chunk: output projection (already natural), normalize,
            # residual, LayerNorm ---
            for i, (off, p) in enumerate(CHUNKS):
                # out_nat[i, h'] = sum_h ctxT[h, i] * WdT[h, h']
                ops = ppm.tile([p, H], f32, tag="p64")
                nc.tensor.matmul(ops[:], ctxT[:, off : off + p], wdt, start=True, stop=True)

                res = wk.tile([p, H], f32, tag=f"res{off}")
                # res = out_nat * r + x
                nc.vector.scalar_tensor_tensor(
                    res[:], ops[:], rcol[i][:], xc[i][:], op0=Alu.mult, op1=Alu.add
                )
                if use_bde:
                    nc.vector.tensor_add(res[:], res[:], bde_b[0:p, :])

                stats = wk.tile([p, 6], f32, tag=f"st{off}")
                nc.vector.bn_stats(stats[:], res[:])
                mv = wk.tile([p, 2], f32, tag=f"mv{off}")
                nc.vector.bn_aggr(mv[:], stats[:])
                # rstd = exp(-0.5*ln(var+eps)); Ln+Exp live in one table set
                lnv = wk.tile([p, 1], f32, tag=f"lnv{off}")
                nc.scalar.activation(lnv[:], mv[:, 1:2], Act.Ln, bias=epsc[0:p, :])
                rstd = wk.tile([p, 1], f32, tag=f"rst{off}")
                nc.scalar.activation(rstd[:], lnv[:], Act.Exp, scale=-0.5)

                y_t = wk.tile([p, H], f32, tag=f"y{off}")
                nc.vector.tensor_scalar(
                    y_t[:], res[:], mv[:, 0:1], rstd[:], op0=Alu.subtract, op1=Alu.mult
                )
                if use_ln:
                    nc.vector.tensor_mul(y_t[:], y_t[:], lnw_b[0:p, :])
                    nc.vector.tensor_add(y_t[:], y_t[:], lnb_b[0:p, :])
                # separate DGE queues so the two output stores issue in parallel
                eng = nc.scalar if i == 0 else nc.sync
                eng.dma_start(out=y_d[off : off + p, :], in_=y_t[:])

    # Compile with the combined Ln+Exp activation-table set preferred, so a
    # single ACT_TABLE_LOAD covers Square/Copy/Exp/Ln (the default greedy
    # selection alternates between the exp-only and ln-only sets: 6 loads,
    # ~7.7us of ACT time).  The set *order* must be preserved — the position
    # in this dict is the act_func_set_id walrus resolves against
    # act_info.json — so instead of reordering, hide this kernel's functions
    # from every other set, forcing the selector onto the combined one at
    # its true index.
    orig_tables = bacc_mod.get_activation_tables
    mine = {Act.Exp, Act.Ln, Act.Square, Act.Copy, Act.Identity}

    def _patched(arch):
        tabs = orig_tables(arch)
        assert "natural_log_exp_and_others" in tabs
        return {
            n: (fs if n == "natural_log_exp_and_others" else fs - mine)
            for n, fs in tabs.items()
        }

    bacc_mod.get_activation_tables = _patched
    try:
        nc.compile()
    finally:
        bacc_mod.get_activation_tables = orig_tables
    return nc


def _prepare(inputs):
    import ml_dtypes

    bf = ml_dtypes.bfloat16
    x = np.ascontiguousarray(np.asarray(inputs["input_tensor"], dtype=np.float32))
    mask = np.ascontiguousarray(np.asarray(inputs["attention_mask"], dtype=np.float32))
    Wq = np.asarray(inputs["Wq"], dtype=np.float32)
    bq = np.asarray(inputs["bq"], dtype=np.float32)
    Wv = np.asarray(inputs["Wv"], dtype=np.float32)
    bv = np.asarray(inputs["bv"], dtype=np.float32)
    Wd = np.asarray(inputs["Wd"], dtype=np.float32)
    bd = np.asarray(inputs["bd"], dtype=np.float32)
    ln_w = np.asarray(inputs["ln_w"], dtype=np.float32)
    ln_b = np.asarray(inputs["ln_b"], dtype=np.float32)
    scale = np.float32(np.asarray(inputs["scale_factor"]).reshape(()))

    use_mask = bool(np.any(mask != 0.0))
    wvals = (mask[:, 0, :] > -10000.0).astype(np.float32)
    use_w = not bool(np.all(wvals == 1.0))
    bde = bd + Wd @ bv  # value bias folded through the output projection
    use_bde = bool(np.any(bde != 0.0))
    use_ln = not (bool(np.all(ln_w == 1.0)) and bool(np.all(ln_b == 0.0)))
    use_bq = bool(np.any(bq != 0.0))

    flags = (use_mask, use_w, use_bde, use_ln, use_bq, float(scale))
    w3 = np.concatenate([Wq.T, Wv.T, Wd.T], axis=1)  # [H, 3H]
    shared = {
        "w3": np.ascontiguousarray(w3).astype(bf),
    }
    if use_bq:
        shared["bqp"] = np.ascontiguousarray((bq * (H ** -0.25)).reshape(H, 1))
    if use_bde:
        shared["bde"] = np.ascontiguousarray(bde.reshape(1, H))
    if use_ln:
        shared["lnw"] = np.ascontiguousarray(ln_w.reshape(1, H))
        shared["lnb"] = np.ascontiguousarray(ln_b.reshape(1, H))

    in_maps = []
    for c in range(N_CORES):
        m = dict(shared)
        m["x"] = np.ascontiguousarray(x[c])
        m["xt"] = np.ascontiguousarray(x[c].T).astype(bf)
        if use_mask:
            m["maskt"] = np.ascontiguousarray(mask[c].T)
        if use_w:
            m["wrow"] = np.ascontiguousarray(wvals[c].reshape(1, L))
        in_maps.append(m)
    return flags, in_maps


def _get_program(flags):
    if flags not in _programs:
        _programs[flags] = _build_program(*flags)
    return _programs[flags]


def kernel(**inputs):
    from concourse.bass_utils import run_bass_kernel_spmd

    flags, in_maps = _prepare(inputs)
    nc = _get_program(flags)
    res = run_bass_kernel_spmd(nc, in_maps, core_ids=list(range(N_CORES)))
    out = np.stack([res.results[c]["y"] for c in range(N_CORES)], axis=0)
    return out.astype(np.float32)


# revision 76
# speedup vs baseline: 1.0508x; 1.0129x over previous
"""DPP attention-3 Trainium2 kernel.

Data-parallel across 8 NeuronCores: one batch element per core; all
weights replicated.  The reference's [B,L,L,L] det_values tensor is never
materialized: since K = s2 @ s2.T is exactly symmetric, the k-reduction of
the 3x3 determinants collapses to

    marginal[i,j] = S0*(d_i d_j - K_ij^2) - d_i a_j - a_i d_j + 2 K_ij A_ij

with  A = K diag(w) K,  a = diag(A),  d = diag(K),  S0 = sum_k w_k d_k.

Everything is computed in the transposed [j, i] orientation (score is
symmetric up to the additive mask, which is fed pre-transposed from the
host), so the exp'd scores are directly the stationary operand of the
context matmul, the softmax denominators fall out of ones-column matmuls,
and the final output projection needs no transpose at all (ctx^T is the
lhsT the natural-orientation matmul wants).

Matmul operands are kept in bf16 (fp32 PSUM accumulation): fp32 matmuls
cost 4 cycles per output row on TRN2 vs 1 for bf16, and the score scale
here (|score| < 1) makes the bf16 rounding negligible (~6e-5 final rel
err measured).  The residual/LayerNorm path stays fp32.
"""

import numpy as np

B, L, H = 8, 160, 64
N_CORES = 8
EPS = 1e-12
CHUNKS = [(0, 128), (128, 32)]  # partition chunks covering L=160

_programs = {}  # (flags..., scale) -> nc


def _build_program(use_mask, use_w, use_bde, use_ln, use_bq, scale):
    import concourse.bass as bass
    import concourse.bacc as bacc_mod
    import concourse.tile as tile
    from concourse import bacc, mybir
    from concourse.masks import make_identity

    f32 = mybir.dt.float32
    bf16 = mybir.dt.bfloat16
    Alu = mybir.AluOpType
    Act = mybir.ActivationFunctionType

    nc = bacc.Bacc(
        "TRN2",
        target_bir_lowering=False,
        debug=False,
        enable_asserts=False,
        num_devices=N_CORES,
    )

    inv_h4 = float(H ** -0.25)

    xt_d = nc.dram_tensor("xt", [H, L], bf16, kind="ExternalInput").ap()
    x_d = nc.dram_tensor("x", [L, H], f32, kind="ExternalInput").ap()
    w3_d = nc.dram_tensor("w3", [H, 3 * H], bf16, kind="ExternalInput").ap()
    bqp_d = None
    if use_bq:
        bqp_d = nc.dram_tensor("bqp", [H, 1], f32, kind="ExternalInput").ap()
    maskt_d = wrow_d = bde_d = lnw_d = lnb_d = None
    if use_mask:
        maskt_d = nc.dram_tensor("maskt", [L, L], f32, kind="ExternalInput").ap()
    if use_w:
        wrow_d = nc.dram_tensor("wrow", [1, L], f32, kind="ExternalInput").ap()
    if use_bde:
        bde_d = nc.dram_tensor("bde", [1, H], f32, kind="ExternalInput").ap()
    if use_ln:
        lnw_d = nc.dram_tensor("lnw", [1, H], f32, kind="ExternalInput").ap()
        lnb_d = nc.dram_tensor("lnb", [1, H], f32, kind="ExternalInput").ap()
    y_d = nc.dram_tensor("y", [L, H], f32, kind="ExternalOutput").ap()

    with tile.TileContext(nc) as tc:
        from contextlib import ExitStack

        with ExitStack() as ctx:
            con = ctx.enter_context(tc.tile_pool(name="con", bufs=1))
            wk = ctx.enter_context(tc.tile_pool(name="wk", bufs=1))
            ppb = ctx.enter_context(tc.tile_pool(name="ppb", bufs=3, space="PSUM"))
            pps = ctx.enter_context(tc.tile_pool(name="pps", bufs=3, space="PSUM"))
            ppm = ctx.enter_context(tc.tile_pool(name="ppm", bufs=2, space="PSUM"))

            # --- inputs; spread descriptor generation across engine DGE
            # queues (a single queue costs ~600ns of issue time per DMA) ---
            xT = con.tile([H, L], bf16)
            nc.scalar.dma_start(out=xT[:], in_=xt_d)
            w3 = con.tile([H, 3 * H], bf16)
            nc.sync.dma_start(out=w3[:], in_=w3_d)
            wqt = w3[:, 0:H]
            wvt = w3[:, H : 2 * H]
            wdt = w3[:, 2 * H : 3 * H]
            bqp = con.tile([H, 1], f32)
            if use_bq:
                nc.sync.dma_start(out=bqp[:], in_=bqp_d)
            else:
                nc.vector.memset(bqp[:], 0.0)
            xc = []
            for i, (off, p) in enumerate(CHUNKS):
                t = con.tile([p, H], f32, tag=f"x{off}")
                eng = nc.sync if i == 0 else nc.gpsimd
                eng.dma_start(out=t[:], in_=x_d[off : off + p, :])
                xc.append(t)

            # --- constants (gpsimd; no deps) ---
            ident_bf = con.tile([128, 128], bf16)
            make_identity(nc, ident_bf[:])
            ones64b = con.tile([H, 1], bf16)
            nc.gpsimd.memset(ones64b[:], 1.0)
            ones128b = con.tile([128, 1], bf16)
            nc.gpsimd.memset(ones128b[:], 1.0)
            onesr = con.tile([1, 128], f32)
            nc.gpsimd.memset(onesr[:], 1.0)
            epsc = con.tile([128, 1], f32)
            nc.gpsimd.memset(epsc[:], EPS)
            ident1 = con.tile([1, 1], f32)
            nc.gpsimd.memset(ident1[:], 1.0)

            masktc = []
            if use_mask:
                for off, p in CHUNKS:
                    t = con.tile([p, L], f32, tag=f"mt{off}")
                    nc.sync.dma_start(out=t[:], in_=maskt_d[off : off + p, :])
                    masktc.append(t)
            if use_w:
                ident = con.tile([128, 128], f32)
                make_identity(nc, ident[:])
                wrow = con.tile([1, L], f32)
                nc.sync.dma_start(out=wrow[:], in_=wrow_d)
            if use_bde:
                bde_r = con.tile([1, H], f32)
                nc.sync.dma_start(out=bde_r[:], in_=bde_d)
            if use_ln:
                lnw_r = con.tile([1, H], f32)
                nc.sync.dma_start(out=lnw_r[:], in_=lnw_d)
                lnb_r = con.tile([1, H], f32)
                nc.sync.dma_start(out=lnb_r[:], in_=lnb_d)

            # pull the single ACT table load off the critical path
            warm = wk.tile([1, 1], f32)
            nc.vector.memset(warm[:], 1.0)
            warm2 = wk.tile([1, 1], f32)
            nc.scalar.copy(warm2[:], warm[:])

            # --- sampler^2 transposed: s2T = Square(invH4*(Wq @ xT) + bq*invH4)
            qT_ps = ppb.tile([H, L], f32, tag="big")
            nc.tensor.matmul(qT_ps[:], wqt, xT[:], start=True, stop=True)
            s2T = wk.tile([H, L], bf16)
            nc.scalar.activation(s2T[:], qT_ps[:], Act.Square, bias=bqp[:], scale=inv_h4)

            # --- K chunks [p, L]; KK taken straight from PSUM so it does not
            # wait on the SBUF cast ---
            Kc = []
            KKc = []
            for i, (off, p) in enumerate(CHUNKS):
                kps = ppb.tile([p, L], f32, tag="big")
                nc.tensor.matmul(kps[:], s2T[:, off : off + p], s2T[:], start=True, stop=True)
                k_sb = wk.tile([p, L], bf16, tag=f"K{off}")
                if i == 0:
                    # DVE: chunk 0's cast gates KK -> a-row, and DVE is free
                    # here while ACT still runs SQUARE
                    nc.vector.tensor_copy(k_sb[:], kps[:])
                else:
                    nc.scalar.copy(k_sb[:], kps[:])
                Kc.append(k_sb)
                kk = wk.tile([p, L], bf16, tag=f"KK{off}")
                nc.vector.tensor_mul(kk[:], kps[:], k_sb[:])
                KKc.append(kk)

            # wK2s early on the ACT queue: every A2s matmul needs both chunks
            wK2s = []
            for i, (off, p) in enumerate(CHUNKS):
                t = wk.tile([p, L], bf16, tag=f"wk2{off}")
                if use_w:
                    pass  # filled in after wcol is built
                else:
                    nc.scalar.mul(t[:], Kc[i][:], -2.0 * scale)
                wK2s.append(t)

            # --- d = diag(K) via s4T = s2T*s2T ---
            s4T = wk.tile([H, L], bf16)
            nc.vector.tensor_mul(s4T[:], s2T[:], s2T[:])
            drow_ps = pps.tile([1, L], f32, tag="small")
            nc.tensor.matmul(drow_ps[:], ones64b[:], s4T[:], start=True, stop=True)
            drow = wk.tile([1, L], bf16)
            S0acc_t = wk.tile([1, 1], f32)
            # ACT copy-cast with accum_out gives S0 = sum_k d_k for free
            nc.scalar.activation(
                drow[:], drow_ps[:], Act.Copy, accum_out=S0acc_t[:]
            )
            dcol_ps = []
            for off, p in CHUNKS:
                dps = pps.tile([p, 1], f32, tag="small")
                nc.tensor.matmul(dps[:], s4T[:, off : off + p], ones64b[:], start=True, stop=True)
                dcol_ps.append(dps)

            wcol = [None, None]
            if use_w:
                for i, (off, p) in enumerate(CHUNKS):
                    wps = pps.tile([p, 1], f32, tag="small")
                    nc.tensor.transpose(wps[:], wrow[0:1, off : off + p], ident[0:1, 0:1])
                    wc = wk.tile([p, 1], f32, tag=f"wc{off}")
                    nc.vector.tensor_copy(wc[:], wps[:])
                    wcol[i] = wc

            # --- a = diag(K diag(w) K) from KK row sums (emitted early: the
            # a-row -> v0 -> Rs chain gates the final score adds) ---
            if use_w:
                for i in range(2):
                    nc.vector.tensor_scalar(
                        KKc[i][:], KKc[i][:], wcol[i][:], None, op0=Alu.mult
                    )
            arow_ps = pps.tile([1, L], f32, tag="small")
            nc.tensor.matmul(arow_ps[:], ones128b[:], KKc[0][:], start=True, stop=False)
            nc.tensor.matmul(arow_ps[:], ones128b[0:32, :], KKc[1][:], start=False, stop=True)
            # scale*a directly off PSUM: the raw a-row is never needed —
            # Rs = d (x) v0 + (s*a) (x) d, with v0 = -S0s*d + s*a
            arow_s = wk.tile([1, L], bf16)
            nc.scalar.mul(arow_s[:], arow_ps[:], scale)

            # S0 = sum_k w_k d_k  (scalar [1,1], fp32)
            if use_w:
                S0_t = wk.tile([1, 1], f32)
                wd_row = wk.tile([1, L], f32)
                nc.vector.tensor_mul(wd_row[:], drow[:], wrow[:])
                nc.vector.reduce_sum(S0_t[:], wd_row[:], axis=mybir.AxisListType.X)
            else:
                S0_t = S0acc_t
            S0s_t = wk.tile([1, 1], f32)
            nc.vector.tensor_scalar(S0s_t[:], S0_t[:], scale, None, op0=Alu.mult)
            nS0s_t = wk.tile([1, 1], f32)
            nc.vector.tensor_scalar(nS0s_t[:], S0_t[:], -scale, None, op0=Alu.mult)
            # scale*S0 broadcast down partition columns (stays in PSUM; the
            # score chain reads it as a per-partition scalar from there)
            S0scol = []
            for off, p in CHUNKS:
                sps = pps.tile([p, 1], f32, tag="small")
                nc.tensor.matmul(sps[:], onesr[0:1, 0:p], S0s_t[:], start=True, stop=True)
                S0scol.append(sps)

            # --- rank-2 factor: v0 = -S0s*d + s*a  (bf16 row) ---
            v0_r = wk.tile([1, L], bf16)
            nc.vector.scalar_tensor_tensor(
                v0_r[:], drow[:], nS0s_t[:], arow_s[:], op0=Alu.mult, op1=Alu.add
            )

            # diag contribution as a matmul operand:
            # dsel[p, f] = -scale*d_p at f == p+off else 0  (bf16)
            dsel = []
            for i, (off, p) in enumerate(CHUNKS):
                dsc = wk.tile([p, 1], f32, tag=f"dsc{off}")
                nc.scalar.mul(dsc[:], dcol_ps[i][:], -scale)
                ds = wk.tile([p, L], bf16, tag=f"dsel{off}")
                nc.gpsimd.affine_select(
                    out=ds[:],
                    in_=dsc[:, 0:1].broadcast_to([p, L]),
                    compare_op=Alu.is_equal,
                    fill=0.0,
                    base=-off,
                    pattern=[[1, L]],
                    channel_multiplier=-1,
                )
                dsel.append(ds)

            # rank-2 part plus the diagonal (identity-stationary matmul);
            # emitted before the A2s group so it does not gate the final adds
            rsc = []
            for i, (off, p) in enumerate(CHUNKS):
                rs = ppm.tile([p, L], f32, tag="p64")
                nc.tensor.matmul(rs[:], drow[0:1, off : off + p], v0_r[:], start=True, stop=False)
                nc.tensor.matmul(rs[:], arow_s[0:1, off : off + p], drow[:], start=False, stop=False)
                nc.tensor.matmul(rs[:], ident_bf[0:p, 0:p], dsel[i][:], start=False, stop=True)
                rsc.append(rs)

            if use_w:
                for i in range(2):
                    nc.vector.tensor_scalar(
                        wK2s[i][:], Kc[i][:], wcol[i][:], -2.0 * scale,
                        op0=Alu.mult, op1=Alu.mult,
                    )

            # --- score + exp per chunk (transposed orientation) ---
            ec = []
            for i, (off, p) in enumerate(CHUNKS):
                a2s = ppb.tile([p, L], f32, tag="big")
                nc.tensor.matmul(a2s[:], Kc[0][:, off : off + p], wK2s[0][:], start=True, stop=False)
                nc.tensor.matmul(a2s[:], Kc[1][:, off : off + p], wK2s[1][:], start=False, stop=True)

                t1 = wk.tile([p, L], f32, tag=f"t1{off}")
                # t1 = S0s*K + A2s
                nc.vector.scalar_tensor_tensor(
                    t1[:], Kc[i][:], S0scol[i][:], a2s[:], op0=Alu.mult, op1=Alu.add
                )
                # t1 = t1 ⊙ K
                nc.vector.tensor_mul(t1[:], t1[:], Kc[i][:])
                # t1 += Rs + diag
                nc.vector.tensor_add(t1[:], t1[:], rsc[i][:])
                if use_mask:
                    nc.vector.tensor_add(t1[:], t1[:], masktc[i][:])
                e = wk.tile([p, L], bf16, tag=f"e{off}")
                nc.scalar.activation(e[:], t1[:], Act.Exp)
                ec.append(e)

            # --- value projection (bf16) ---
            Vh = []
            for i, (off, p) in enumerate(CHUNKS):
                vps = ppm.tile([p, H], f32, tag="p64")
                nc.tensor.matmul(vps[:], xT[:, off : off + p], wvt, start=True, stop=True)
                vh = wk.tile([p, H], bf16, tag=f"vh{off}")
                nc.scalar.copy(vh[:], vps[:])
                Vh.append(vh)

            # softmax denominators Z as a row (tiny all-ones stationary),
            # transposed to columns, then per-partition reciprocals (a DVE
            # reciprocal along the free dim would serialize: ~7 cyc/element)
            zrow_ps = pps.tile([1, L], f32, tag="small")
            nc.tensor.matmul(zrow_ps[:], ones128b[:], ec[0][:], start=True, stop=False)
            nc.tensor.matmul(zrow_ps[:], ones128b[0:32, :], ec[1][:], start=False, stop=True)
            zrow = wk.tile([1, L], f32)
            nc.vector.tensor_copy(zrow[:], zrow_ps[:])
            rcol = []
            for off, p in CHUNKS:
                zps = pps.tile([p, 1], f32, tag="small")
                nc.tensor.transpose(zps[:], zrow[0:1, off : off + p], ident1[:])
                rc = wk.tile([p, 1], f32, tag=f"rc{off}")
                nc.vector.reciprocal(rc[:], zps[:])
                rcol.append(rc)

            # ctxT [H, L] = V^T e^T
            ctxT_ps = ppb.tile([H, L], f32, tag="big")
            nc.tensor.matmul(ctxT_ps[:], Vh[0][:], ec[0][:], start=True, stop=False)
            nc.tensor.matmul(ctxT_ps[:], Vh[1][:], ec[1][:], start=False, stop=True)
            ctxT = wk.tile([H, L], bf16)
            # split the copy across engines so chunk 0's output matmul starts
            # early and the halves convert in parallel
            nc.scalar.copy(ctxT[:, 0:128], ctxT_ps[:, 0:128])
            nc.vector.tensor_copy(ctxT[:, 128:160], ctxT_ps[:, 128:160])

            if use_bde:
                bde_ps = ppm.tile([128, H], f32, tag="p64")
                nc.tensor.matmul(bde_ps[:], onesr[:], bde_r[:], start=True, stop=True)
                bde_b = wk.tile([128, H], f32)
                nc.vector.tensor_copy(bde_b[:], bde_ps[:])
            if use_ln:
                lnw_ps = ppm.tile([128, H], f32, tag="p64")
                nc.tensor.matmul(lnw_ps[:], onesr[:], lnw_r[:], start=True, stop=True)
                lnw_b = wk.tile([128, H], f32)
                nc.vector.tensor_copy(lnw_b[:], lnw_ps[:])
                lnb_ps = ppm.tile([128, H], f32, tag="p64")
                nc.tensor.matmul(lnb_ps[:], onesr[:], lnb_r[:], start=True, stop=True)
                lnb_b = wk.tile([128, H], f32)
                nc.vector.tensor_copy(lnb_b[:], lnb_ps[:])

            # --- per chunk: output projection (already natural), normalize,
            # residual, LayerNorm ---
            for i, (off, p) in enumerate(CHUNKS):
                # out_nat[i, h'] = sum_h ctxT[h, i] * WdT[h, h']
                ops = ppm.tile([p, H], f32, tag="p64")
                nc.tensor.matmul(ops[:], ctxT[:, off : off + p], wdt, start=True, stop=True)

                res = wk.tile([p, H], f32, tag=f"res{off}")
                # res = out_nat * r + x
                nc.vector.scalar_tensor_tensor(
                    res[:], ops[:], rcol[i][:], xc[i][:], op0=Alu.mult, op1=Alu.add
                )
                if use_bde:
                    nc.vector.tensor_add(res[:], res[:], bde_b[0:p, :])

                stats = wk.tile([p, 6], f32, tag=f"st{off}")
                nc.vector.bn_stats(stats[:], res[:])
                mv = wk.tile([p, 2], f32, tag=f"mv{off}")
                nc.vector.bn_aggr(mv[:], stats[:])
                # rstd = exp(-0.5*ln(var+eps)); Ln+Exp live in one table set
                lnv = wk.tile([p, 1], f32, tag=f"lnv{off}")
                nc.scalar.activation(lnv[:], mv[:, 1:2], Act.Ln, bias=epsc[0:p, :])
                rstd = wk.tile([p, 1], f32, tag=f"rst{off}")
                nc.scalar.activation(rstd[:], lnv[:], Act.Exp, scale=-0.5)

                y_t = wk.tile([p, H], f32, tag=f"y{off}")
                nc.vector.tensor_scalar(
                    y_t[:], res[:], mv[:, 0:1], rstd[:], op0=Alu.subtract, op1=Alu.mult
                )
                if use_ln:
                    nc.vector.tensor_mul(y_t[:], y_t[:], lnw_b[0:p, :])
                    nc.vector.tensor_add(y_t[:], y_t[:], lnb_b[0:p, :])
                # separate DGE queues so the two output stores issue in parallel
                eng = nc.scalar if i == 0 else nc.sync
                eng.dma_start(out=y_d[off : off + p, :], in_=y_t[:])

    # Compile with the combined Ln+Exp activation-table set preferred, so a
    # single ACT_TABLE_LOAD covers Square/Copy/Exp/Ln (the default greedy
    # selection alternates between the exp-only and ln-only sets: 6 loads,
    # ~7.7us of ACT time).  The set *order* must be preserved — the position
    # in this dict is the act_func_set_id walrus resolves against
    # act_info.json — so instead of reordering, hide this kernel's functions
    # from every other set, forcing the selector onto the combined one at
    # its true index.
    orig_tables = bacc_mod.get_activation_tables
    mine = {Act.Exp, Act.Ln, Act.Square, Act.Copy, Act.Identity}

    def _patched(arch):
        tabs = orig_tables(arch)
        assert "natural_log_exp_and_others" in tabs
        return {
            n: (fs if n == "natural_log_exp_and_others" else fs - mine)
            for n, fs in tabs.items()
        }

    bacc_mod.get_activation_tables = _patched
    try:
        nc.compile()
    finally:
        bacc_mod.get_activation_tables = orig_tables
    return nc


def _prepare(inputs):
    import ml_dtypes

    bf = ml_dtypes.bfloat16
    x = np.ascontiguousarray(np.asarray(inputs["input_tensor"], dtype=np.float32))
    mask = np.ascontiguousarray(np.asarray(inputs["attention_mask"], dtype=np.float32))
    Wq = np.asarray(inputs["Wq"], dtype=np.float32)
    bq = np.asarray(inputs["bq"], dtype=np.float32)
    Wv = np.asarray(inputs["Wv"], dtype=np.float32)
    bv = np.asarray(inputs["bv"], dtype=np.float32)
    Wd = np.asarray(inputs["Wd"], dtype=np.float32)
    bd = np.asarray(inputs["bd"], dtype=np.float32)
    ln_w = np.asarray(inputs["ln_w"], dtype=np.float32)
    ln_b = np.asarray(inputs["ln_b"], dtype=np.float32)
    scale = np.float32(np.asarray(inputs["scale_factor"]).reshape(()))

    use_mask = bool(np.any(mask != 0.0))
    wvals = (mask[:, 0, :] > -10000.0).astype(np.float32)
    use_w = not bool(np.all(wvals == 1.0))
    bde = bd + Wd @ bv  # value bias folded through the output projection
    use_bde = bool(np.any(bde != 0.0))
    use_ln = not (bool(np.all(ln_w == 1.0)) and bool(np.all(ln_b == 0.0)))
    use_bq = bool(np.any(bq != 0.0))

    flags = (use_mask, use_w, use_bde, use_ln, use_bq, float(scale))
    w3 = np.concatenate([Wq.T, Wv.T, Wd.T], axis=1)  # [H, 3H]
    shared = {
        "w3": np.ascontiguousarray(w3).astype(bf),
    }
    if use_bq:
        shared["bqp"] = np.ascontiguousarray((bq * (H ** -0.25)).reshape(H, 1))
    if use_bde:
        shared["bde"] = np.ascontiguousarray(bde.reshape(1, H))
    if use_ln:
        shared["lnw"] = np.ascontiguousarray(ln_w.reshape(1, H))
        shared["lnb"] = np.ascontiguousarray(ln_b.reshape(1, H))

    in_maps = []
    for c in range(N_CORES):
        m = dict(shared)
        m["x"] = np.ascontiguousarray(x[c])
        m["xt"] = np.ascontiguousarray(x[c].T).astype(bf)
        if use_mask:
            m["maskt"] = np.ascontiguousarray(mask[c].T)
        if use_w:
            m["wrow"] = np.ascontiguousarray(wvals[c].reshape(1, L))
        in_maps.append(m)
    return flags, in_maps


def _get_program(flags):
    if flags not in _programs:
        _programs[flags] = _build_program(*flags)
    return _programs[flags]


def kernel(**inputs):
    from concourse.bass_utils import run_bass_kernel_spmd

    flags, in_maps = _prepare(inputs)
    nc = _get_program(flags)
    res = run_bass_kernel_spmd(nc, in_maps, core_ids=list(range(N_CORES)))
    out = np.stack([res.results[c]["y"] for c in range(N_CORES)], axis=0)
    return out.astype(np.float32)
